# revision 7
# baseline (speedup 1.0000x reference)
"""Trainium2 Bass kernel for nn_AttentionBasedClassifier1.

Reference computation (B=64, P=128, D=1024):
    c = concat(h, q, -1)                      # (B, P, 2D)
    s = c @ W_att[:, 0] + b_att               # (B, P)
    p = softmax(s, axis=1)
    y = sum_i p[b,i] * c[b,i,:]               # (B, 2D)
    out = y @ W_fc + b_fc                     # (B, 1)

Algebraic restructure used here:
    * b_att shifts every logit equally -> cancels in softmax; dropped.
    * e = exp(s) unnormalized; y_un[b,:] = sum_i e[b,i] c[b,i,:],
      den[b] = sum_i e[b,i].  Then out = (y_un @ W_fc)/den + b_fc.
      The final 2048-length dot + division + bias run on host (tiny).

Device work per core (data-parallel over batch, 8 batches/core):
    * one fused DVE tensor_tensor_reduce per batch computes
      s[b] = sum_d c[b,:,d] * W_att[d]   (c streamed once, 1MB/batch)
    * ACT exp -> column b of a zeroed (128,8) "masked E" tile
    * TensorE matmuls with the masked E as the stationary operand
      accumulate row b of a shared (8, 2048) PSUM region + (8,1) denom
      (other rows get +0 because the mask zeroes other columns)
    * ACT copies PSUM->SBUF once at the end; DMA out (8,2048)+(8,1).

The kernel is DMA-bound: 8MB of HBM reads per core (~23us at ~358GB/s).
"""

import os
import sys

for _p in ("/opt/trn_rl_repo", "/root/.axon_site/_ro/trn_rl_repo"):
    if os.path.isdir(_p) and _p not in sys.path:
        sys.path.append(_p)

import numpy as np
from contextlib import ExitStack

import concourse.bass as bass
import concourse.tile as tile
from concourse import mybir
from concourse.bass_utils import run_bass_kernel_spmd

N_CORES = 8
B, P, D = 64, 128, 1024
D2 = 2 * D
BPC = B // N_CORES  # batches per core
F32 = mybir.dt.float32


def _split_multi_waits(nc, max_waits=1):
    """walrus (CoreV3) rejects instructions carrying several sync-waits;
    hoist extras onto same-engine NOPs placed just before the instruction."""
    for f in nc.m.functions:
        for blk in f.blocks:
            insts = blk.instructions
            i = 0
            while i < len(insts):
                inst = insts[i]
                si = getattr(inst, "sync_info", None)
                if si is not None and si.on_wait and len(si.on_wait) > max_waits:
                    extra = list(si.on_wait[:-max_waits])
                    keep = list(si.on_wait[-max_waits:])
                    nops = []
                    for w in extra:
                        nop = mybir.InstNoOp(
                            name=nc.get_next_instruction_name(), ins=[], outs=[]
                        )
                        nop.engine = inst.engine
                        nop.sync_info = mybir.SyncInfo(on_wait=[w], on_update=[])
                        nc.register_instruction(nop)
                        nops.append(nop)
                    si.on_wait = keep
                    for j, nop in enumerate(nops):
                        insts.insert(i + j, nop)
                    i += len(nops)
                i += 1


def build_nc(reps=1, data_bufs=5, loop_iters=1):
    """Build the per-core Bass module. `reps` statically repeats the whole
    pipeline; `loop_iters` wraps it in a hardware For_i loop (same
    inputs/outputs each iteration) for steady-state timing measurements."""
    nc = bass.Bass()
    c_in = nc.declare_dram_parameter("c", [BPC, P, D2], F32, isOutput=False)
    wa_in = nc.declare_dram_parameter("wa", [D2, 1], F32, isOutput=False)
    y_out = nc.declare_dram_parameter("y", [BPC, D2], F32, isOutput=True)
    den_out = nc.declare_dram_parameter("den", [BPC, 1], F32, isOutput=True)

    with ExitStack() as ctx:
        tc = ctx.enter_context(tile.TileContext(nc))
        const = ctx.enter_context(tc.tile_pool(name="const", bufs=1))
        data = ctx.enter_context(tc.tile_pool(name="data", bufs=data_bufs))
        scols = ctx.enter_context(tc.tile_pool(name="scols", bufs=4))
        epool = ctx.enter_context(tc.tile_pool(name="epool", bufs=4))
        opool = ctx.enter_context(tc.tile_pool(name="opool", bufs=2))
        psum = ctx.enter_context(tc.tile_pool(name="psum", bufs=1, space="PSUM"))

        # W_att broadcast to all 128 partitions via a stride-0 DMA source AP.
        wa_b = const.tile([P, D2], F32)
        wa_ap = wa_in[:, :]
        wa_bcast = bass.AP(tensor=wa_ap.tensor, offset=wa_ap.offset, ap=[[0, P], [1, D2]])
        nc.gpsimd.dma_start(out=wa_b[:, :], in_=wa_bcast)
        ones = const.tile([P, 1], F32)
        nc.vector.memset(ones[:, :], 1.0)
        dummy = const.tile([P, 1], F32)  # garbage sink for TTR elementwise out

        loop_cm = tc.For_i(0, loop_iters, 1) if loop_iters > 1 else None
        if loop_cm is not None:
            ctx.enter_context(loop_cm)

        for _rep in range(reps):
            ch = [
                psum.tile([BPC, 512], F32, name=f"ch{k}", tag=f"ch{k}")
                for k in range(4)
            ]
            dn = psum.tile([BPC, 1], F32, name="dn", tag="dn")
            for b in range(BPC):
                ct = data.tile([P, D2], F32)
                nc.sync.dma_start(out=ct[:, :], in_=c_in[b])
                s_col = scols.tile([P, 1], F32)
                nc.vector.scalar_tensor_tensor(
                    out=dummy.broadcast_to([P, D2]),
                    in0=ct[:, :],
                    scalar=1.0,
                    in1=wa_b[:, :],
                    op0=mybir.AluOpType.mult,
                    op1=mybir.AluOpType.mult,
                    accum_out=s_col[:, :],
                )
                eb = epool.tile([P, BPC], F32)
                nc.gpsimd.memset(eb[:, :], 0.0)
                nc.scalar.activation(
                    out=eb[:, b : b + 1],
                    in_=s_col[:, :],
                    func=mybir.ActivationFunctionType.Exp,
                )
                for k in range(4):
                    nc.tensor.matmul(
                        ch[k][:, :],
                        eb[:, :],
                        ct[:, 512 * k : 512 * (k + 1)],
                        start=(b == 0),
                        stop=(b == BPC - 1),
                        skip_group_check=True,
                    )
                nc.tensor.matmul(
                    dn[:, :],
                    eb[:, :],
                    ones[:, :],
                    start=(b == 0),
                    stop=(b == BPC - 1),
                    skip_group_check=True,
                )
            y_sb = opool.tile([BPC, D2], F32)
            dn_sb = opool.tile([BPC, 1], F32)
            for k in range(4):
                nc.scalar.copy(out=y_sb[:, 512 * k : 512 * (k + 1)], in_=ch[k][:, :])
            nc.scalar.copy(out=dn_sb[:, :], in_=dn[:, :])
            nc.sync.dma_start(out=y_out[:, :], in_=y_sb[:, :])
            nc.sync.dma_start(out=den_out[:, :], in_=dn_sb[:, :])

    _split_multi_waits(nc)
    return nc


_NC_CACHE = {}


def _get_nc(reps=1):
    if reps not in _NC_CACHE:
        _NC_CACHE[reps] = build_nc(reps)
    return _NC_CACHE[reps]


def make_in_maps(h, q, W_att):
    c = np.concatenate([h, q], axis=-1).astype(np.float32)  # (B, P, 2D)
    wa = np.ascontiguousarray(W_att, dtype=np.float32)
    return [
        {"c": np.ascontiguousarray(c[i * BPC : (i + 1) * BPC]), "wa": wa}
        for i in range(N_CORES)
    ]


def finish_on_host(results, W_fc, b_fc):
    y = np.concatenate([results[i]["y"] for i in range(N_CORES)], axis=0)  # (B, 2D)
    den = np.concatenate([results[i]["den"] for i in range(N_CORES)], axis=0)  # (B,1)
    num = y.astype(np.float64) @ W_fc.astype(np.float64)  # (B, 1)
    out = num / den.astype(np.float64) + np.asarray(b_fc, np.float64)
    return out.astype(np.float32)


def kernel(h, q, W_att, b_att, W_fc, b_fc):
    nc = _get_nc(reps=1)
    in_maps = make_in_maps(h, q, W_att)
    res = run_bass_kernel_spmd(nc, in_maps, list(range(N_CORES)))
    return finish_on_host(res.results, W_fc, b_fc)
